# revision 1
# baseline (speedup 1.0000x reference)
"""LongFormer dilated-window attention block on 8 trn2 NeuronCores.

Sharding: 8 cores = 4 dilation residues x 2 sequence halves. Query q
attends keys q + 4*j - 512 (j=0..256), i.e. only keys with the same
residue mod DIL=4. De-interleaving by residue turns the dilated window
into a contiguous +-128 sliding window in "residue space". Each core
gets a zero-padded [512, 512] slice of x: its 256 owned rows plus a
128-row halo on each side (all in residue space), so no communication
is needed. All matmuls run as float32r (fp32 data, reduced-precision
fp22 PE reads) which is full-rate on the PE for moving dims >= 256.

Pipeline: head-pair j's scores run while pair j-1's p@v, softmax
normalization and FFN1 partial accumulation execute, keeping the PE
dense. PSUM budget (8 banks): 3 rotating transient banks + 3 p@v
accumulator banks + 2 FFN1 accumulator banks.
"""

import sys

if "/opt/trn_rl_repo" not in sys.path:
    sys.path.insert(0, "/opt/trn_rl_repo")

import numpy as np

N_CORES = 8
S, E, H, FEAT = 2048, 512, 8, 64
DIL = 4
SC = 256      # owned queries per core (residue space)
EXT = 512     # ext rows per core (owned + 128 halo each side)

_CACHE = {}


def _build_nc():
    import concourse.bacc as bacc
    import concourse.tile as tile
    import concourse.mybir as mybir
    import concourse.bass as bass
    from concourse.masks import make_identity

    dt = mybir.dt
    f32 = dt.float32
    f32r = dt.float32r
    Alu = mybir.AluOpType
    Act = mybir.ActivationFunctionType

    nc = bacc.Bacc("TRN2", target_bir_lowering=False, debug=False,
                   num_devices=N_CORES)

    # ---- DRAM I/O ----
    xe_d = nc.dram_tensor("xe", [EXT, E], f32, kind="ExternalInput").ap()
    wq_d = nc.dram_tensor("wq", [E, E], f32r, kind="ExternalInput").ap()
    wk_d = nc.dram_tensor("wk", [E, E], f32r, kind="ExternalInput").ap()
    wv_d = nc.dram_tensor("wv", [E, E], f32r, kind="ExternalInput").ap()
    w1_d = nc.dram_tensor("w1", [E, E], f32r, kind="ExternalInput").ap()
    w2_d = nc.dram_tensor("w2", [E, E], f32r, kind="ExternalInput").ap()
    b1_d = nc.dram_tensor("b1", [E], f32, kind="ExternalInput").ap()
    b2_d = nc.dram_tensor("b2", [E], f32, kind="ExternalInput").ap()
    out_d = nc.dram_tensor("out", [SC, E], f32, kind="ExternalOutput").ap()

    # affine_select band masks, p viewed as [kc, hh, s, qs]: keep iff iota>=0
    AFF = [
        # c_abs=0: kc - qs - 256*s >= 0
        dict(base=0, channel_multiplier=1, pattern=[[0, 2], [-256, 2], [-1, 128]]),
        # c_abs=1: kc - qs + 128 - 128*s >= 0
        dict(base=128, channel_multiplier=1, pattern=[[0, 2], [-128, 2], [-1, 128]]),
        # c_abs=2: -kc + qs + 128*s >= 0
        dict(base=0, channel_multiplier=-1, pattern=[[0, 2], [128, 2], [1, 128]]),
        # c_abs=3: -kc + qs - 256 + 256*s >= 0
        dict(base=-256, channel_multiplier=-1, pattern=[[0, 2], [256, 2], [1, 128]]),
    ]

    with tile.TileContext(nc) as tc:
        with (
            tc.tile_pool(name="singles", bufs=1) as singles,
            tc.tile_pool(name="ptiles", bufs=8) as ptiles,
            tc.tile_pool(name="rec", bufs=3) as recpool,
            tc.tile_pool(name="ps_big", bufs=4, space="PSUM") as ps_big,
            tc.tile_pool(name="ps_pv", bufs=2, space="PSUM") as ps_pv,
            tc.tile_pool(name="ps_ffn", bufs=1, space="PSUM") as ps_ffn,
        ):
            # ---- persistent SBUF tiles ----
            xe_nat = singles.tile([128, 4, E], f32)     # [p, seq_chunk, e]
            wq_sb = singles.tile([128, 4, E], f32r)     # [p, e_chunk, f]
            wk_sb = singles.tile([128, 4, E], f32r)
            wv_sb = singles.tile([128, 4, E], f32r)
            w1_sb = singles.tile([128, 4, E], f32r)
            w2_sb = singles.tile([128, 4, E], f32r)
            b1_bc = singles.tile([128, E], f32)
            b2_bc = singles.tile([128, E], f32)
            ident = singles.tile([128, 128], f32)
            ones8 = singles.tile([128, 64], f32r)       # 0.125 everywhere
            xeT = singles.tile([128, 4, EXT], f32r)     # [p, e_chunk, seq]
            qT = singles.tile([128, 4, SC], f32r)       # [p, f_chunk, q]
            kT = singles.tile([128, 4, EXT], f32r)      # [p, f_chunk, key]
            v_sb = singles.tile([128, 4, H, 65], f32r)  # [p, seq_chunk, h, f+1]
            x1T = singles.tile([128, 4, SC], f32r)      # [p, e_chunk, q]
            f_sb = singles.tile([128, 2, E], f32)       # [p(q), qc, f1]
            fT = singles.tile([128, 4, SC], f32r)       # [p, f1_chunk, q]
            out_sb = singles.tile([128, 2, E], f32)

            # ---- input DMAs (chunked, in consumption order) ----
            for o in range(4):
                nc.sync.dma_start(xe_nat[:, o, :],
                                  xe_d[128 * o:128 * o + 128, :])

            def w_cols(w_sb, w_d, j):
                nc.sync.dma_start(
                    w_sb[:, :, 128 * j:128 * j + 128],
                    w_d[:, 128 * j:128 * j + 128].rearrange(
                        "(o p) f -> p o f", p=128))

            w_cols(wq_sb, wq_d, 0)
            w_cols(wk_sb, wk_d, 0)
            for o in range(4):
                nc.sync.dma_start(wv_sb[:, o, :],
                                  wv_d[128 * o:128 * o + 128, :])
            for j in range(1, 4):
                w_cols(wq_sb, wq_d, j)
                w_cols(wk_sb, wk_d, j)
            for o in range(4):
                nc.sync.dma_start(w1_sb[:, o, :],
                                  w1_d[128 * o:128 * o + 128, :])
            for o in range(4):
                nc.sync.dma_start(w2_sb[:, o, :],
                                  w2_d[128 * o:128 * o + 128, :])
            # ---- constants ----
            make_identity(nc, ident)
            # f32r tiles cannot be memset directly (invalid ISA); build
            # constants in f32 scratch and round via DVE copies.
            sc_c = singles.tile([128, 128], f32)
            nc.vector.memset(sc_c[:, 0:64], 0.125)
            nc.vector.memset(sc_c[:, 64:96], 1.0)
            nc.vector.tensor_copy(out=ones8[:], in_=sc_c[:, 0:64])
            nc.vector.tensor_copy(
                out=v_sb[:, :, :, 64:65],
                in_=sc_c[:, 64:96].rearrange("p (a b c) -> p a b c", a=4, b=8))
            # shift64[k, 64+k] = 1 for k<64: a PE matmul with this as lhsT
            # moves SBUF partitions 0:64 to PSUM partitions 64:128.
            sc_s = singles.tile([128, 128], f32)
            nc.gpsimd.memset(sc_s[:], 0.0)
            nc.gpsimd.affine_select(
                out=sc_s[:], in_=sc_s[:], compare_op=Alu.not_equal,
                fill=1.0, base=64, channel_multiplier=1, pattern=[[-1, 128]])
            shift64 = singles.tile([128, 128], f32r)
            nc.vector.tensor_copy(out=shift64[:], in_=sc_s[:])

            # bias rows broadcast across partitions via 0-stride DMA read;
            # emitted after the constants so the slow SWDGE broadcasts do
            # not block make_identity (and with it all PE transposes) in
            # the in-order GpSimd queue. Biases are first used ~60us in.
            nc.gpsimd.dma_start(
                out=b1_bc[:],
                in_=bass.AP(tensor=b1_d.tensor, offset=b1_d.offset,
                            ap=[[0, 128]] + b1_d.ap))
            nc.gpsimd.dma_start(
                out=b2_bc[:],
                in_=bass.AP(tensor=b2_d.tensor, offset=b2_d.offset,
                            ap=[[0, 128]] + b2_d.ap))

            # ---- build xeT via PE transposes ----
            for sc in range(4):
                for eo in range(4):
                    tp = ps_big.tile([128, 128], f32, tag="big", name="tp")
                    nc.tensor.transpose(
                        tp[:], xe_nat[:, sc, 128 * eo:128 * eo + 128],
                        ident[:])
                    nc.vector.tensor_copy(
                        out=xeT[:, eo, 128 * sc:128 * sc + 128], in_=tp[:])

            # FFN1 accumulators, filled as x1T chunks are produced
            f_ps = [ps_ffn.tile([128, E], f32, tag=f"f{qc}", name=f"f{qc}")
                    for qc in range(2)]

            def emit_proj(j):
                # project pair j's qT/kT feature chunk (heads 2j, 2j+1)
                qp = ps_big.tile([128, SC], f32, tag="big", name="qp")
                for ke in range(4):
                    nc.tensor.matmul(
                        qp[:], wq_sb[:, ke, 128 * j:128 * j + 128],
                        xeT[:, ke, 128:384],
                        start=(ke == 0), stop=(ke == 3))
                nc.scalar.copy(out=qT[:, j, :], in_=qp[:])
                kp = ps_big.tile([128, EXT], f32, tag="big", name="kp")
                for ke in range(4):
                    nc.tensor.matmul(
                        kp[:], wk_sb[:, ke, 128 * j:128 * j + 128],
                        xeT[:, ke, :],
                        start=(ke == 0), stop=(ke == 3))
                nc.scalar.copy(out=kT[:, j, :], in_=kp[:])

            def emit_scores(j):
                p_list = []
                for ca in range(4):
                    # one psum bank per matmul group: the PE faults if two
                    # groups touch the same 2KB bank, even disjoint columns
                    sps = []
                    for hh in range(2):
                        o = 64 * hh
                        sp = ps_big.tile([128, SC], f32, tag="big", name="sp")
                        nc.tensor.matmul(
                            sp[:],
                            kT[o:o + 64, j, 128 * ca:128 * ca + 128],
                            qT[o:o + 64, j, :],
                            start=True, stop=True)
                        sps.append(sp)
                    p_t = ptiles.tile([128, 2, 2, 128], f32r, tag="p",
                                      name="p_t")
                    for hh in range(2):
                        nc.scalar.activation(
                            out=p_t[:, hh, :, :].rearrange(
                                "p a b -> p (a b)"),
                            in_=sps[hh][:], func=Act.Exp)
                    nc.gpsimd.affine_select(
                        out=p_t[:], in_=p_t[:],
                        compare_op=Alu.is_ge, fill=0.0, **AFF[ca])
                    p_list.append(p_t)
                return p_list

            pvps = {}
            recs = {}

            def emit_pv(j, p_list):
                # p@v accumulation + (DVE) denominator reciprocal
                rec = recpool.tile([128, EXT], f32r, tag="rec")
                for hh in range(2):
                    h = 2 * j + hh
                    pvp = ps_pv.tile([65, SC], f32, tag="pv", name="pvp")
                    for ca in range(4):
                        nc.tensor.matmul(
                            pvp[:], v_sb[:, ca, h, :],
                            p_list[ca][:, hh, :, :].rearrange(
                                "p a b -> p (a b)"),
                            start=(ca == 0), stop=(ca == 3))
                    with nc.allow_low_precision(reason="denominator recip"):
                        nc.vector.reciprocal(
                            out=rec[64:65, 256 * hh:256 * hh + 256],
                            in_=pvp[64:65, :])
                    pvps[(j, hh)] = pvp
                recs[j] = rec

            bcss = {}

            def emit_norm_a(j):
                # C': broadcast 0.125/denom for both heads with one N=512
                # outer-product matmul; scale even head into x1T and stage
                # the odd head's scaled tile (tmp_o) for the shift matmul
                rec = recs.pop(j)
                bc = ps_big.tile([128, EXT], f32, tag="big", name="bc")
                nc.tensor.matmul(
                    bc[0:64, :], ones8[64:65, :], rec[64:65, :],
                    start=True, stop=True)
                bcs = recpool.tile([128, EXT], f32, tag="bcs")
                nc.scalar.copy(out=bcs[0:64, :], in_=bc[0:64, :])
                pvp0 = pvps.pop((j, 0))
                nc.vector.tensor_mul(
                    out=x1T[0:64, j, :], in0=pvp0[0:64, :],
                    in1=bcs[0:64, 0:256])
                pvp1 = pvps.pop((j, 1))
                tmp_o = recpool.tile([128, SC], f32r, tag="tmpo")
                nc.vector.tensor_mul(
                    out=tmp_o[0:64, :], in0=pvp1[0:64, :],
                    in1=bcs[0:64, 256:512])
                bcss[j] = tmp_o

            def emit_norm_b(j):
                # C'': odd head belongs at x1T partitions 64:128; no engine
                # shifts partitions, so bounce through a PE matmul against
                # the constant shift64 matrix. Then fold the finished x1T
                # chunk into the FFN1 accumulation.
                tmp_o = bcss.pop(j)
                shp = ps_big.tile([128, SC], f32, tag="big", name="shp")
                nc.tensor.matmul(
                    shp[:], shift64[0:64, :], tmp_o[0:64, :],
                    start=True, stop=True)
                nc.vector.tensor_copy(
                    out=x1T[64:128, j, :], in_=shp[64:128, :])
                for qc in range(2):
                    nc.tensor.matmul(
                        f_ps[qc][:], x1T[:, j, 128 * qc:128 * qc + 128],
                        w1_sb[:, j, :],
                        start=(j == 0), stop=(j == 3))

            # ---- attention, software pipelined ----
            # stages per pair j: A(j)=proj+scores, B(j)=p@v+recip,
            # C'(j)=denominator broadcast+scaling, C''(j)=partition shift
            # + FFN1 fold. Each stage runs over a stage behind its
            # producer so DVE/ACT latency (recip, copies, muls) never
            # stalls the in-order PE queue.
            emit_proj(0)
            scores = [emit_scores(0)]
            # v natural: [seq_chunk, f]; strided copy into v_sb [.., h, 0:64]
            for sc in range(4):
                vp = ps_big.tile([128, E], f32, tag="big", name="vp")
                for ke in range(4):
                    nc.tensor.matmul(
                        vp[:], xeT[:, ke, 128 * sc:128 * sc + 128],
                        wv_sb[:, ke, :],
                        start=(ke == 0), stop=(ke == 3))
                nc.scalar.copy(
                    out=v_sb[:, sc, :, 0:64],
                    in_=vp[:].rearrange("p (h f) -> p h f", h=H))
            emit_proj(1)
            scores.append(emit_scores(1))
            emit_pv(0, scores[0])
            emit_proj(2)
            scores.append(emit_scores(2))
            emit_norm_a(0)
            emit_pv(1, scores[1])
            emit_proj(3)
            scores.append(emit_scores(3))
            emit_norm_b(0)
            emit_norm_a(1)
            emit_pv(2, scores[2])
            emit_norm_b(1)
            emit_norm_a(2)
            emit_pv(3, scores[3])
            emit_norm_b(2)
            emit_norm_a(3)
            emit_norm_b(3)

            # ---- FFN1 epilogue: f = relu(psum + b1), transpose to fT ----
            for qc in range(2):
                nc.vector.tensor_add(
                    out=f_sb[:, qc, :], in0=f_ps[qc][:], in1=b1_bc[:])
                nc.vector.tensor_scalar_max(
                    out=f_sb[:, qc, :], in0=f_sb[:, qc, :], scalar1=0.0)
                for u in range(4):
                    tp = ps_big.tile([128, 128], f32, tag="big", name="tp2")
                    nc.tensor.transpose(
                        tp[:], f_sb[:, qc, 128 * u:128 * u + 128], ident[:])
                    nc.vector.tensor_copy(
                        out=fT[:, u, 128 * qc:128 * qc + 128], in_=tp[:])

            # ---- FFN2 + relu + residual ----
            for qc in range(2):
                gp = ps_big.tile([128, E], f32, tag="big", name="gp")
                for u in range(4):
                    nc.tensor.matmul(
                        gp[:], fT[:, u, 128 * qc:128 * qc + 128],
                        w2_sb[:, u, :],
                        start=(u == 0), stop=(u == 3))
                nc.vector.tensor_add(
                    out=out_sb[:, qc, :], in0=gp[:], in1=b2_bc[:])
                nc.vector.tensor_scalar_max(
                    out=out_sb[:, qc, :], in0=out_sb[:, qc, :], scalar1=0.0)
                nc.vector.tensor_add(
                    out=out_sb[:, qc, :], in0=out_sb[:, qc, :],
                    in1=xe_nat[:, 1 + qc, :])
                nc.sync.dma_start(
                    out_d[128 * qc:128 * qc + 128, :], out_sb[:, qc, :])

    nc.compile()
    return nc


def _get_nc():
    if "nc" not in _CACHE:
        _CACHE["nc"] = _build_nc()
    return _CACHE["nc"]


def _shard_inputs(x, Wq, Wk, Wv, W1, b1, W2, b2):
    x2 = np.ascontiguousarray(np.asarray(x, dtype=np.float32).reshape(S, E))
    ws = {
        "wq": np.ascontiguousarray(np.asarray(Wq, np.float32).T),
        "wk": np.ascontiguousarray(np.asarray(Wk, np.float32).T),
        "wv": np.ascontiguousarray(np.asarray(Wv, np.float32).T),
        "w1": np.ascontiguousarray(np.asarray(W1, np.float32).T),
        "w2": np.ascontiguousarray(np.asarray(W2, np.float32).T),
        "b1": np.ascontiguousarray(np.asarray(b1, np.float32)),
        "b2": np.ascontiguousarray(np.asarray(b2, np.float32)),
    }
    in_maps = []
    for c in range(N_CORES):
        r_, half = c >> 1, c & 1
        eidx = np.arange(256 * half - 128, 256 * half + 384)
        valid = (eidx >= 0) & (eidx < S // DIL)
        xe = np.zeros((EXT, E), np.float32)
        xe[valid] = x2[DIL * eidx[valid] + r_]
        in_maps.append({"xe": xe, **ws})
    return in_maps


def _gather_outputs(results):
    out = np.zeros((S, E), np.float32)
    for c in range(N_CORES):
        r_, half = c >> 1, c & 1
        i = np.arange(256 * half, 256 * half + SC)
        out[DIL * i + r_] = results[c]["out"]
    return out.reshape(1, S, E)


def run(inputs, trace=False, tmpdir=None):
    from concourse import bass_utils
    nc = _get_nc()
    in_maps = _shard_inputs(**inputs)
    res = bass_utils.run_bass_kernel_spmd(
        nc, in_maps, list(range(N_CORES)), trace=trace, tmpdir=tmpdir)
    return _gather_outputs(res.results), res


def kernel(x, Wq, Wk, Wv, W1, b1, W2, b2):
    out, _ = run(dict(x=x, Wq=Wq, Wk=Wk, Wv=Wv, W1=W1, b1=b1, W2=W2, b2=b2))
    return out



# revision 30
# speedup vs baseline: 1.4012x; 1.4012x over previous
"""LongFormer dilated-window attention block on 8 trn2 NeuronCores.

Sharding: 8 cores = 4 dilation residues x 2 sequence halves. Query q
attends keys q + 4*j - 512 (j=0..256), i.e. only keys with the same
residue mod DIL=4. De-interleaving by residue turns the dilated window
into a contiguous +-128 sliding window in "residue space". Each core
gets a zero-padded [512, 512] slice of x: its 256 owned rows plus a
128-row halo on each side (all in residue space), so no communication
is needed.

v2: bf16 data path (halved HBM traffic, DMA-transposed x), band-sparse
scores (the two dead key chunks per query sub-block are skipped),
single-group one-bank score tiles so exp runs as one ACT op per tile,
reciprocal_approx_fast + a 2-partition selector matmul for the softmax
denominators (odd head lands at PSUM partitions 64:128 directly), FFN
computed transposed with rank-1 bias matmuls, and a PE warm-up burst
during the input DMA so real matmuls run at 2.4 GHz from the start.
"""

import sys

if "/opt/trn_rl_repo" not in sys.path:
    sys.path.insert(0, "/opt/trn_rl_repo")

import numpy as np

N_CORES = 8
S, E, H, FEAT = 2048, 512, 8, 64
DIL = 4
SC = 256      # owned queries per core (residue space)
EXT = 512     # ext rows per core (owned + 128 halo each side)
WARM_MM = 36  # dummy matmuls to keep the PE HAM window busy at start

_CACHE = {}


def _build_nc():
    import concourse.bacc as bacc
    import concourse.tile as tile
    import concourse.mybir as mybir
    import concourse.bass as bass

    dt = mybir.dt
    f32 = dt.float32
    f32r = dt.float32r
    bf16 = dt.bfloat16
    Alu = mybir.AluOpType
    Act = mybir.ActivationFunctionType

    nc = bacc.Bacc("TRN2", target_bir_lowering=False, debug=False,
                   num_devices=N_CORES)

    # ---- DRAM I/O ----
    xe_d = nc.dram_tensor("xe", [EXT, E], bf16, kind="ExternalInput").ap()
    wq_d = nc.dram_tensor("wq", [E, E], bf16, kind="ExternalInput").ap()
    wk_d = nc.dram_tensor("wk", [E, E], bf16, kind="ExternalInput").ap()
    wv_d = nc.dram_tensor("wv", [E, E], bf16, kind="ExternalInput").ap()
    w1_d = nc.dram_tensor("w1", [E, E], bf16, kind="ExternalInput").ap()
    w2_d = nc.dram_tensor("w2", [E, E], bf16, kind="ExternalInput").ap()
    b1_d = nc.dram_tensor("b1", [1, E], f32r, kind="ExternalInput").ap()
    b2_d = nc.dram_tensor("b2", [1, E], f32r, kind="ExternalInput").ap()
    out_d = nc.dram_tensor("out", [SC, E], f32, kind="ExternalOutput").ap()
    import os as _os0
    DBG = bool(_os0.environ.get("KDBG"))
    dbg = {}
    if DBG:
        for nm, shp, dt_ in [
            ("dqT", [128, 4, SC], bf16), ("dkT", [128, 4, EXT], bf16),
            ("dvsb", [128, 4, H, 128], bf16),
            ("dpA", [128, 3, 128], bf16), ("dpB", [128, 3, 128], bf16),
            ("dbcs", [128, SC], f32), ("dx1T", [128, 4, SC], bf16),
            ("dfT", [128, 4, SC], bf16),
        ]:
            dbg[nm] = nc.dram_tensor(nm, shp, dt_,
                                     kind="ExternalOutput").ap()

    with tile.TileContext(nc) as tc:
        with (
            tc.tile_pool(name="singles", bufs=1) as singles,
            tc.tile_pool(name="ptiles", bufs=8) as ptiles,
            tc.tile_pool(name="rec", bufs=2) as recpool,
            tc.tile_pool(name="bcs", bufs=2) as bcspool,
            tc.tile_pool(name="ps_sc", bufs=3, space="PSUM") as ps_sc,
            tc.tile_pool(name="ps_big", bufs=2, space="PSUM") as ps_big,
            tc.tile_pool(name="ps_pv", bufs=2, space="PSUM") as ps_pv,
            tc.tile_pool(name="ps_ffn", bufs=1, space="PSUM") as ps_ffn,
        ):
            # ---- persistent SBUF tiles ----
            xeT = singles.tile([128, 4, EXT], bf16)     # [p, e_chunk, seq]
            xe_nat = singles.tile([128, 2, E], bf16)    # owned rows, natural
            wq_sb = singles.tile([128, 4, E], bf16)     # [p, e_chunk, f]
            wk_sb = singles.tile([128, 4, E], bf16)
            wv_sb = singles.tile([128, 4, E], bf16)
            w1_sb = singles.tile([128, 4, E], bf16)
            w2_sb = singles.tile([128, 4, E], bf16)
            b1_sb = singles.tile([1, E], f32r)
            b2_sb = singles.tile([1, E], f32r)
            ones_row = singles.tile([1, SC], f32r)      # 1.0, rank-1 biases
            onesP = singles.tile([128, 128], f32r)      # denom broadcast rows
            qT = singles.tile([128, 4, SC], bf16)       # [p, f_chunk, q]
            kT = singles.tile([128, 4, EXT], bf16)      # [p, f_chunk, key]
            # [p, seq_chunk, h, 128]: even heads [v(64), 8.0, junk]; odd
            # heads [8.0, zeros(63), v(64)] so one 128-col lhsT covers all
            # output partitions (denominator at partition 0, pv at 64:128)
            v_sb = singles.tile([128, 4, H, 128], bf16)
            x1T = singles.tile([128, 4, SC], bf16)      # [p, e_chunk, q]
            fT = singles.tile([128, 4, SC], bf16)       # [p, f1_chunk, q]
            out_sb = singles.tile([128, 2, E], f32)

            # ---- input DMAs (in consumption order) ----
            # x transposed via the DMA XBAR, one 128-col chunk at a time
            import os as _os
            if _os.environ.get("NO_DMAT"):
                for o in range(4):
                    nc.sync.dma_start(
                        xeT[:, o, :],
                        xe_d[:, 128 * o:128 * o + 128].rearrange(
                            "a b -> b a"))
            else:
                for o in range(4):
                    nc.sync.dma_start_transpose(
                        xeT[:, o, :], xe_d[:, 128 * o:128 * o + 128])

            def w_cols(w_sb, w_d, j):
                nc.sync.dma_start(
                    w_sb[:, :, 128 * j:128 * j + 128],
                    w_d[:, 128 * j:128 * j + 128].rearrange(
                        "(o p) f -> p o f", p=128))

            w_cols(wq_sb, wq_d, 0)
            w_cols(wk_sb, wk_d, 0)
            for o in range(4):
                nc.sync.dma_start(wv_sb[:, o, :],
                                  wv_d[128 * o:128 * o + 128, :])
            for j in range(1, 4):
                w_cols(wq_sb, wq_d, j)
                w_cols(wk_sb, wk_d, j)
            for o in range(4):
                nc.sync.dma_start(w1_sb[:, o, :],
                                  w1_d[128 * o:128 * o + 128, :])
            nc.sync.dma_start(b1_sb[:], b1_d[:])
            nc.sync.dma_start(b2_sb[:], b2_d[:])
            for o in range(4):
                nc.sync.dma_start(w2_sb[:, o, :],
                                  w2_d[128 * o:128 * o + 128, :])
            for qc in range(2):
                nc.sync.dma_start(xe_nat[:, qc, :],
                                  xe_d[128 + 128 * qc:256 + 128 * qc, :])

            # ---- constants ----
            # f32r tiles cannot be memset directly; build in f32 scratch
            # and round via DVE copies.
            scf = singles.tile([128, 128], f32)
            nc.vector.memset(scf[:], 0.0)
            nc.vector.memset(scf[64:65, :], 1.0)
            nc.vector.memset(scf[0:1, :], 1.0)
            nc.vector.tensor_copy(out=onesP[:], in_=scf[:])
            sco = singles.tile([1, SC], f32)
            nc.vector.memset(sco[:], 1.0)
            nc.vector.tensor_copy(out=ones_row[:], in_=sco[:])
            # softmax denominator helpers: 8.0 (folds the 1/sqrt(64))
            if DBG:
                nc.vector.memset(v_sb[:, :, 0::2, 65:128], 0.0)
            nc.vector.memset(v_sb[:, :, 0::2, 64:65], 8.0)
            nc.vector.memset(v_sb[:, :, 1::2, 0:64], 0.0)
            nc.vector.memset(v_sb[:, :, 1::2, 0:1], 8.0)

            # ---- PE warm-up: keep the HAM activity window busy while the
            # input DMAs land so the real matmuls run at 2.4 GHz ----
            if not _os.environ.get("NO_WARM"):
                wp = ps_big.tile([128, 512], f32, tag="big", name="wp")
                for i in range(WARM_MM):
                    nc.tensor.matmul(wp[:, 0:128], onesP[:], onesP[:],
                                     start=True, stop=True)
                junk = singles.tile([1, 1], f32)
                nc.vector.tensor_copy(out=junk[:], in_=wp[0:1, 0:1])

            # ---- attention stages ----
            def emit_proj(j):
                qp = ps_big.tile([128, 512], f32, tag="big", name="qp")
                for ke in range(4):
                    nc.tensor.matmul(
                        qp[:, 0:SC], wq_sb[:, ke, 128 * j:128 * j + 128],
                        xeT[:, ke, 128:384],
                        start=(ke == 0), stop=(ke == 3))
                nc.vector.tensor_copy(out=qT[:, j, :], in_=qp[:, 0:SC])
                kp = ps_big.tile([128, 512], f32, tag="big", name="kp")
                for ke in range(4):
                    nc.tensor.matmul(
                        kp[:], wk_sb[:, ke, 128 * j:128 * j + 128],
                        xeT[:, ke, :],
                        start=(ke == 0), stop=(ke == 3))
                nc.vector.tensor_copy(out=kT[:, j, :], in_=kp[:])

            def emit_scores(j):
                # per head: two one-bank score tiles, band-sparse:
                #   tileA = [ca0|s0, ca1|s1, ca1|s0]  (tri, tri, full)
                #   tileB = [ca3|s1, ca2|s0, ca2|s1]  (tri, tri, full)
                # the two triangles of a tile share one affine predicate.
                p_js = []
                for hh in range(2):
                    o = 64 * hh
                    k_ = lambda ca: kT[o:o + 64, j, 128 * ca:128 * ca + 128]
                    q_ = lambda s0, s1: qT[o:o + 64, j, 128 * s0:128 * s1]
                    spA = ps_sc.tile([128, 512], f32, tag="sc", name="spA")
                    nc.tensor.matmul(spA[:, 0:128], k_(0), q_(0, 1),
                                     start=True, stop=False)
                    nc.tensor.matmul(spA[:, 128:256], k_(1), q_(1, 2),
                                     start=False, stop=False)
                    nc.tensor.matmul(spA[:, 256:384], k_(1), q_(0, 1),
                                     start=False, stop=True)
                    pa = ptiles.tile([128, 3, 128], bf16, tag="p", name="pa")
                    nc.scalar.activation(
                        out=pa[:].rearrange("p a b -> p (a b)"),
                        in_=spA[:, 0:384], func=Act.Exp)
                    nc.gpsimd.affine_select(
                        out=pa[:, 0:2, :], in_=pa[:, 0:2, :],
                        compare_op=Alu.is_ge, fill=0.0,
                        base=0, channel_multiplier=1,
                        pattern=[[0, 2], [-1, 128]])
                    spB = ps_sc.tile([128, 512], f32, tag="sc", name="spB")
                    nc.tensor.matmul(spB[:, 0:128], k_(3), q_(1, 2),
                                     start=True, stop=False)
                    nc.tensor.matmul(spB[:, 128:384], k_(2), q_(0, 2),
                                     start=False, stop=True)
                    pb = ptiles.tile([128, 3, 128], bf16, tag="p", name="pb")
                    nc.scalar.activation(
                        out=pb[:].rearrange("p a b -> p (a b)"),
                        in_=spB[:, 0:384], func=Act.Exp)
                    nc.gpsimd.affine_select(
                        out=pb[:, 0:2, :], in_=pb[:, 0:2, :],
                        compare_op=Alu.is_ge, fill=0.0,
                        base=0, channel_multiplier=-1,
                        pattern=[[0, 2], [1, 128]])
                    if DBG and j == 0 and hh == 0:
                        nc.sync.dma_start(dbg["dpA"][:], pa[:])
                        nc.sync.dma_start(dbg["dpB"][:], pb[:])
                    p_js.append((pa, pb))
                return p_js

            def emit_vproj():
                for sc_ in range(4):
                    vp = ps_big.tile([128, 512], f32, tag="big", name="vp")
                    for ke in range(4):
                        nc.tensor.matmul(
                            vp[:], xeT[:, ke, 128 * sc_:128 * sc_ + 128],
                            wv_sb[:, ke, :],
                            start=(ke == 0), stop=(ke == 3))
                    vh = vp[:].rearrange("p (h f) -> p h f", h=H)
                    nc.scalar.copy(out=v_sb[:, sc_, 0::2, 0:64],
                                   in_=vh[:, 0::2, :])
                    nc.scalar.copy(out=v_sb[:, sc_, 1::2, 64:128],
                                   in_=vh[:, 1::2, :])

            pvps = {}
            recs = {}
            bcss = {}

            def emit_pv(j, p_js):
                # p@v accumulation. Even head: out at partitions 0:64 with
                # the 8.0 column of v landing 8*denom at partition 64. Odd
                # head: out at partitions 64:128 (PE base must be 0/32/64),
                # denominator via extra 1-column matmuls into partition 0
                # of the same bank/group.
                for hh in range(2):
                    pa, pb = p_js[hh]
                    pvp = ps_pv.tile([128, 512], f32, tag="pv", name="pvp")
                    if hh == 0:
                        vs = lambda ca: v_sb[:, ca, 2 * j, 0:65]
                        ov = pvp[0:65, 0:SC]
                    else:
                        vs = lambda ca: v_sb[:, ca, 2 * j + 1, :]
                        ov = pvp[0:128, 0:SC]
                    pbm = pb[:, 1:3, :].rearrange("p a b -> p (a b)")
                    mms = [
                        (ov[:, 0:256], vs(2), pbm),
                        (ov[:, 0:128], vs(0), pa[:, 0, :]),
                        (ov[:, 128:256], vs(3), pb[:, 0, :]),
                        (ov[:, 0:128], vs(1), pa[:, 2, :]),
                        (ov[:, 128:256], vs(1), pa[:, 1, :]),
                    ]
                    for i, (o_, l_, r_) in enumerate(mms):
                        nc.tensor.matmul(o_, l_, r_, start=(i == 0),
                                         stop=(i == len(mms) - 1))
                    pvps[(j, hh)] = pvp

            def emit_norm_a(j):
                # move the 8*denom rows to SBUF (rounding to f32r), rank-1
                # broadcast them across partitions (one group, first
                # matmul's start clears the whole bank), then reciprocal
                # straight out of PSUM into the f32 scale tile.
                rec = recpool.tile([128, EXT], f32r, tag="rec")
                nc.vector.tensor_copy(out=rec[64:65, 0:SC],
                                      in_=pvps[(j, 0)][64:65, 0:SC])
                nc.vector.tensor_copy(out=rec[0:1, 0:SC],
                                      in_=pvps[(j, 1)][0:1, 0:SC])
                recs[j] = rec
                bc = ps_big.tile([128, 512], f32, tag="big", name="bc")
                nc.tensor.matmul(bc[:, 0:SC], onesP[64:65, :],
                                 rec[64:65, 0:SC], start=True, stop=False)
                nc.tensor.matmul(bc[:, SC:2 * SC], onesP[0:1, :],
                                 rec[0:1, 0:SC], start=False, stop=True)
                bct = bcspool.tile([128, SC], f32, tag="bct")
                nc.scalar.copy(out=bct[0:64, :], in_=bc[0:64, 0:SC])
                nc.scalar.copy(out=bct[64:128, :], in_=bc[64:128, SC:2 * SC])
                bcs = bcspool.tile([128, SC], f32, tag="bcs")
                with nc.allow_low_precision(reason="softmax denominator"):
                    nc.vector.reciprocal_approx_fast(
                        out=bcs[:, :], in_=bct[:, :])
                if DBG and j == 0:
                    nc.sync.dma_start(dbg["dbcs"][:], bcs[:])
                bcss[j] = bcs

            def emit_norm_b(j):
                bcs = bcss.pop(j)
                recs.pop(j)
                pvp0 = pvps.pop((j, 0))
                nc.vector.tensor_mul(
                    out=x1T[0:64, j, :], in0=pvp0[0:64, 0:SC],
                    in1=bcs[0:64, :])
                pvp1 = pvps.pop((j, 1))
                nc.vector.tensor_mul(
                    out=x1T[64:128, j, :], in0=pvp1[64:128, 0:SC],
                    in1=bcs[64:128, :])

            # ---- attention, software pipelined ----
            emit_proj(0)
            scores = [emit_scores(0)]
            emit_vproj()
            emit_proj(1)
            scores.append(emit_scores(1))
            emit_pv(0, scores[0])
            emit_norm_a(0)
            emit_proj(2)
            scores.append(emit_scores(2))
            emit_norm_b(0)
            emit_pv(1, scores[1])
            emit_norm_a(1)
            emit_proj(3)
            scores.append(emit_scores(3))
            emit_norm_b(1)
            emit_pv(2, scores[2])
            emit_norm_a(2)
            emit_norm_b(2)
            emit_pv(3, scores[3])
            emit_norm_a(3)
            emit_norm_b(3)

            # ---- FFN1 (transposed): fT = relu(W1 @ x1T + b1) ----
            for u in range(4):
                fp = ps_ffn.tile([128, 512], f32, tag="f", name="fp")
                for eo in range(4):
                    nc.tensor.matmul(
                        fp[:, 0:SC], w1_sb[:, eo, 128 * u:128 * u + 128],
                        x1T[:, eo, :],
                        start=(eo == 0), stop=False)
                nc.tensor.matmul(
                    fp[:, 0:SC], b1_sb[0:1, 128 * u:128 * u + 128],
                    ones_row[0:1, :],
                    start=False, stop=True)
                nc.scalar.activation(out=fT[:, u, :], in_=fp[:, 0:SC],
                                     func=Act.Relu)

            # ---- FFN2 + relu + residual ----
            for qc in range(2):
                gp = ps_big.tile([128, 512], f32, tag="big", name="gp")
                for u in range(4):
                    nc.tensor.matmul(
                        gp[:], fT[:, u, 128 * qc:128 * qc + 128],
                        w2_sb[:, u, :],
                        start=(u == 0), stop=False)
                nc.tensor.matmul(gp[:], ones_row[0:1, 0:128], b2_sb[0:1, :],
                                 start=False, stop=True)
                nc.vector.scalar_tensor_tensor(
                    out=out_sb[:, qc, :], in0=gp[:], scalar=0.0,
                    in1=xe_nat[:, qc, :], op0=Alu.max, op1=Alu.add)
                nc.sync.dma_start(
                    out_d[128 * qc:128 * qc + 128, :], out_sb[:, qc, :])
            if DBG:
                nc.sync.dma_start(dbg["dqT"][:], qT[:])
                nc.sync.dma_start(dbg["dkT"][:], kT[:])
                nc.sync.dma_start(dbg["dvsb"][:], v_sb[:])
                nc.sync.dma_start(dbg["dx1T"][:], x1T[:])
                nc.sync.dma_start(dbg["dfT"][:], fT[:])

    nc.compile()
    return nc


def _get_nc():
    if "nc" not in _CACHE:
        _CACHE["nc"] = _build_nc()
    return _CACHE["nc"]


def _shard_inputs(x, Wq, Wk, Wv, W1, b1, W2, b2):
    import ml_dtypes
    bf = ml_dtypes.bfloat16
    x2 = np.ascontiguousarray(np.asarray(x, dtype=np.float32).reshape(S, E))
    ws = {
        "wq": np.ascontiguousarray(np.asarray(Wq, np.float32).T.astype(bf)),
        "wk": np.ascontiguousarray(np.asarray(Wk, np.float32).T.astype(bf)),
        "wv": np.ascontiguousarray(np.asarray(Wv, np.float32).T.astype(bf)),
        "w1": np.ascontiguousarray(np.asarray(W1, np.float32).T.astype(bf)),
        "w2": np.ascontiguousarray(np.asarray(W2, np.float32).T.astype(bf)),
        "b1": np.ascontiguousarray(np.asarray(b1, np.float32).reshape(1, E)),
        "b2": np.ascontiguousarray(np.asarray(b2, np.float32).reshape(1, E)),
    }
    in_maps = []
    for c in range(N_CORES):
        r_, half = c >> 1, c & 1
        eidx = np.arange(256 * half - 128, 256 * half + 384)
        valid = (eidx >= 0) & (eidx < S // DIL)
        xe = np.zeros((EXT, E), np.float32)
        xe[valid] = x2[DIL * eidx[valid] + r_]
        in_maps.append({"xe": np.ascontiguousarray(xe.astype(bf)), **ws})
    return in_maps


def _gather_outputs(results):
    out = np.zeros((S, E), np.float32)
    for c in range(N_CORES):
        r_, half = c >> 1, c & 1
        i = np.arange(256 * half, 256 * half + SC)
        out[DIL * i + r_] = results[c]["out"]
    return out.reshape(1, S, E)


def run(inputs, trace=False, tmpdir=None):
    from concourse import bass_utils
    nc = _get_nc()
    in_maps = _shard_inputs(**inputs)
    res = bass_utils.run_bass_kernel_spmd(
        nc, in_maps, list(range(N_CORES)), trace=trace, tmpdir=tmpdir)
    return _gather_outputs(res.results), res


def kernel(x, Wq, Wk, Wv, W1, b1, W2, b2):
    out, _ = run(dict(x=x, Wq=Wq, Wk=Wk, Wv=Wv, W1=W1, b1=b1, W2=W2, b2=b2))
    return out
